# revision 1
# baseline (speedup 1.0000x reference)
"""Trainium2 Bass kernel for nn_ButterflyFFT (Monarch butterfly, N=4096, B=8192).

Math (per batch row b, viewing x[b] as a 64x64 matrix X with X[p,k]=x[b,p*64+k]):
  stage 1: for each column k: Y[:,k] = w1c[k] @ X[:,k]       (64x64 complex, X real)
  stage 2: for each row    l: Z[l,:] = w2c[l] @ Y[l,:]       (64x64 complex)
  output:  out[b, s*64+l] = Z[l,s]                            (complex64)

Device pipeline per core (B_core=1024, supertiles of BT=256):
  1. gather-DMA x -> T1[(h,p), (b0,k)] fp16 (cast in SWDGE DMA)
  2. stage 1, data-stationary fp16 matmuls: out (b, q2) -> G[b, ch, q*128+c*64+r]
  3. PE transpose per (l, ch): G-slice (b, (c r)) -> T2 (rc, b)
  4. stage 2, weights-stationary fp16 matmuls: O2 (c's*64+s, b)
  5. DMA out fp16 (l, cs, b); host reassembles complex64.
"""

import numpy as np

N = 4096
B = 8192
NCORES = 8
B_CORE = B // NCORES  # 1024
BT = 256              # supertile batch
NT = B_CORE // BT     # 4 supertiles
F16 = np.float16


def _build_host_weights(w1_bfly: np.ndarray, w2_bfly: np.ndarray):
    """W1all[64h+p, k*128 + c*64 + q] = w1_bfly[k,q,p,c]  (dup across h)
       W2all[c*64+r, l*128 + c'*64 + s] = stage-2 complex-matmul real form."""
    w1 = w1_bfly.astype(np.float32)              # (k, q, p, c)
    W1 = np.transpose(w1, (2, 0, 3, 1))          # (p, k, c, q)
    W1 = W1.reshape(64, 64 * 128).astype(F16)    # [p, k*128 + c*64 + q]
    W1all = np.concatenate([W1, W1], axis=0)     # dup rows for h=0/1

    w2r = w2_bfly[..., 0].astype(np.float32)     # (l, s, r)
    w2i = w2_bfly[..., 1].astype(np.float32)
    W2 = np.empty((2, 64, 64, 2, 64), dtype=np.float32)  # [c, r, l, c', s]
    W2[0, :, :, 0, :] = np.transpose(w2r, (2, 0, 1))     # rows r,    out re:  w2_re
    W2[1, :, :, 0, :] = -np.transpose(w2i, (2, 0, 1))    # rows 64+r, out re: -w2_im
    W2[0, :, :, 1, :] = np.transpose(w2i, (2, 0, 1))     # rows r,    out im:  w2_im
    W2[1, :, :, 1, :] = np.transpose(w2r, (2, 0, 1))     # rows 64+r, out im:  w2_re
    W2all = W2.reshape(128, 64 * 128).astype(F16)        # [c*64+r, l*128 + c'*64 + s]
    return np.ascontiguousarray(W1all), np.ascontiguousarray(W2all)


def build_bass(repeat=1):
    import concourse.bacc as bacc
    import concourse.mybir as mybir
    import concourse.tile as tile

    f16 = mybir.dt.float16
    f32 = mybir.dt.float32

    nc = bacc.Bacc("TRN2", target_bir_lowering=False)
    x = nc.dram_tensor("x", [B_CORE, N], f32, kind="ExternalInput")
    w1 = nc.dram_tensor("w1", [128, 64 * 128], f16, kind="ExternalInput")
    w2 = nc.dram_tensor("w2", [128, 64 * 128], f16, kind="ExternalInput")
    iddram = nc.dram_tensor("ident", [128, 128], f16, kind="ExternalInput")
    out = nc.dram_tensor("out", [64, 128, B_CORE], f16, kind="ExternalOutput")

    x_v = x[:, :].rearrange("(t h b0) (p k) -> t h p b0 k", h=2, b0=BT // 2, p=64)
    out_v = out[:, :, :].rearrange("L cs (t b) -> t cs L b", b=BT)

    with tile.TileContext(nc) as tc:
        with (
            tc.tile_pool(name="const", bufs=1) as constp,
            tc.tile_pool(name="t1", bufs=3) as t1p,
            tc.tile_pool(name="g", bufs=2) as gp,
            tc.tile_pool(name="t2s", bufs=6) as t2p,
            tc.tile_pool(name="outs", bufs=6) as outp,
            tc.tile_pool(name="po1", bufs=2, space="PSUM") as po1,
            tc.tile_pool(name="pt2", bufs=2, space="PSUM") as pt2,
            tc.tile_pool(name="po2", bufs=2, space="PSUM") as po2,
        ):
            # prefetch the first supertile's input before the weight tables
            T1_first = t1p.tile([128, (BT // 2) * 64], f16, tag="t1")
            T1f_4d = T1_first[:].rearrange("(h p) (b0 k) -> h p b0 k", h=2, k=64)
            for h in range(2):
                for kh in range(2):
                    nc.gpsimd.dma_start(
                        T1f_4d[h][:, :, kh * 32:(kh + 1) * 32],
                        x_v[0][h][:, :, kh * 32:(kh + 1) * 32])
            W1t = constp.tile([128, 64 * 128], f16)
            nc.sync.dma_start(W1t[:], w1[:, :])
            ident = constp.tile([128, 128], f16)
            nc.sync.dma_start(ident[:], iddram[:, :])
            W2t = constp.tile([128, 64 * 128], f16)
            nc.sync.dma_start(W2t[:], w2[:, :])
            W1t_v = W1t[:].rearrange("(h p) f -> h p f", h=2)

            from contextlib import nullcontext
            rep_ctx = tc.For_i(0, repeat, 1) if repeat > 1 else nullcontext()
            with rep_ctx:
                for t in range(NT):
                    # ---- load T1[(h,p), (b0,k)] with fp32->fp16 cast (SWDGE) ----
                    if t == 0 and repeat == 1:
                        T1_4d = T1f_4d
                    else:
                        T1 = t1p.tile([128, (BT // 2) * 64], f16, tag="t1")
                        T1_4d = T1[:].rearrange("(h p) (b0 k) -> h p b0 k", h=2, k=64)
                        for h in range(2):
                            for kh in range(2):
                                nc.gpsimd.dma_start(
                                    T1_4d[h][:, :, kh * 32:(kh + 1) * 32],
                                    x_v[t][h][:, :, kh * 32:(kh + 1) * 32])

                    # ---- stage 1 (data-stationary): G[b, ch, q*128+c*64+r] ----
                    G = gp.tile([128, 2, 64 * 128], f16)
                    G_5d = G[:].rearrange("B ch (q c r) -> B ch q c r", q=64, c=2)
                    for ch in range(2):
                        for kg2 in range(8):
                            O1 = po1.tile([128, 8, 128], f32)
                            for ksub in range(8):
                                k = kg2 * 8 + ksub
                                nc.tensor.matmul(
                                    O1[:, ksub, :],
                                    T1_4d[ch][:, :, k],                  # (64 p, 128 b0)
                                    W1t_v[ch][:, k * 128:(k + 1) * 128], # (64 p, 128 q2)
                                    start=True, stop=True,
                                )
                            # evac + cast f32->f16, (ksub,c,q)->(q,c,ksub)
                            src = O1[:].rearrange(
                                "B ksub (c q) -> B q c ksub", c=2)
                            dst = G_5d[:, ch, :, :, kg2 * 8:(kg2 + 1) * 8]
                            if kg2 % 4 == 3:
                                nc.vector.tensor_copy(dst, src)
                            else:
                                nc.scalar.copy(dst, src)

                    # ---- stage 2, l in quads: PE transposes -> T2s; pairs of mms ----
                    OUTS = None
                    T2s = None
                    for l0 in range(0, 64, 2):
                        grp = (t * 64 + l0) // 8
                        if l0 % 8 == 0:
                            OUTS = outp.tile([128, 8, BT], f16)
                        if l0 % 4 == 0:
                            Pt2 = pt2.tile([128, 8, 128], f16)
                            for lp in range(4):
                                l = l0 + lp
                                for ch in range(2):
                                    nc.tensor.transpose(
                                        Pt2[:, lp * 2 + ch, :],
                                        G[:, ch, l * 128:(l + 1) * 128], ident[:]
                                    )
                            T2s = t2p.tile([128, 4, 256], f16)
                            nc.vector.tensor_copy(T2s[:], Pt2[:])

                        O2 = po2.tile([128, 2, BT], f32)
                        for lp in range(2):
                            l = l0 + lp
                            nc.tensor.matmul(
                                O2[:, lp, :], W2t[:, l * 128:(l + 1) * 128],
                                T2s[:, l % 4, :],
                                start=True, stop=True,
                            )
                        if grp % 2 == 0:
                            nc.scalar.copy(OUTS[:, l0 % 8:l0 % 8 + 2, :], O2[:])
                        else:
                            nc.vector.tensor_copy(OUTS[:, l0 % 8:l0 % 8 + 2, :], O2[:])

                        if l0 % 8 == 6:
                            nc.sync.dma_start(out_v[t][:, l0 - 6:l0 + 2, :], OUTS[:])
    nc.compile()
    return nc


def _assemble_core(o: np.ndarray) -> np.ndarray:
    # o: (64 l, 128 cs, B_CORE) f16, cs = c*64+s  ->  (B_CORE, 4096) complex64
    a = o.reshape(64, 2, 64, B_CORE)                         # (l, c, s, b)
    a = np.ascontiguousarray(np.transpose(a, (3, 2, 0, 1)))  # (b, s, l, c)
    return a.astype(np.float32).view(np.complex64).reshape(B_CORE, N)


def kernel(x, w1_bfly, w2_bfly, perm, _trace=False):
    from concourse.bass_utils import run_bass_kernel_spmd

    x = np.asarray(x, dtype=np.float32)
    w1_bfly = np.asarray(w1_bfly, dtype=np.float32)
    w2_bfly = np.asarray(w2_bfly, dtype=np.float32)

    W1all, W2all = _build_host_weights(w1_bfly, w2_bfly)
    ident = np.eye(128, dtype=F16)
    nc = build_bass()
    in_maps = [
        {
            "x": np.ascontiguousarray(x[i * B_CORE:(i + 1) * B_CORE]),
            "w1": W1all,
            "w2": W2all,
            "ident": ident,
        }
        for i in range(NCORES)
    ]
    res = run_bass_kernel_spmd(
        nc, in_maps, core_ids=list(range(NCORES)), trace=_trace
    )
    outs = [_assemble_core(r["out"]) for r in res.results]
    full = np.concatenate(outs, axis=0)
    if _trace:
        return full, res
    return full



# revision 18
# speedup vs baseline: 1.0524x; 1.0524x over previous
"""Trainium2 Bass kernel for nn_ButterflyFFT (Monarch butterfly, N=4096, B=8192).

Math (per batch row b, with x[b] viewed as X[p,k] = x[b, p*64+k]):
  stage 1: out1[b,k,q] = sum_p w1c[k,q,p] X[p,k]          (complex, X real)
  stage 2: out2[b,l,s] = sum_k w2c[l,s,k] out1[b,k,l]     (complex)
  output:  out[b, s*64+l] = out2[b,l,s]

Device pipeline per core (B_core=1024, supertiles BT=256 = 2 halves h of 128):
  1. contiguous SWDGE cast-DMA x (fp32) -> XS[b0, (p k)] fp16
  2. input transpose on PE: XS[:, (p,dk)@j] -> T1[(p dk), b0] per k-pair j
     (partitions (p,dk)-interleaved; matches host weight row order)
  3. stage 1, k-pair block-diagonal matmuls: out O1[b0, (dk c q)] per (h,j)
  4. middle transpose on PE per (h,l): G[b0,(c k)] -> T2[(c k), b0]
  5. stage 2 weights-stationary: O2[(c' s), (h b0)] per l
  6. one output DMA per 8 l's; host reassembles complex64.
"""

import numpy as np

N = 4096
B = 8192
NCORES = 8
B_CORE = B // NCORES  # 1024
BT = 256              # supertile batch
NT = B_CORE // BT     # 4 supertiles
F16 = np.float16


def _build_host_weights(w1_bfly: np.ndarray, w2_bfly: np.ndarray):
    """W1T[p*2+dk, j*256 + dk'*128 + c*64 + q] = w1[2j+dk', q, p, c] * (dk==dk')
       W2T[c*64+k, l*128 + c'*64 + s] = stage-2 complex-matmul real form."""
    w1 = w1_bfly.astype(np.float32)                # (k, q, p, c)
    w1r = w1.reshape(32, 2, 64, 64, 2)             # (j, dk, q, p, c)
    A = np.zeros((64, 2, 32, 2, 2, 64), np.float32)  # (p, dk, j, dk', c, q)
    for dk in range(2):
        # (j, q, p, c) -> (p, j, c, q)
        A[:, dk, :, dk, :, :] = np.transpose(w1r[:, dk], (2, 0, 3, 1))
    W1T = A.reshape(128, 64 * 128).astype(F16)

    w2r = w2_bfly[..., 0].astype(np.float32)       # (l, s, k)
    w2i = w2_bfly[..., 1].astype(np.float32)
    Bm = np.empty((2, 64, 64, 2, 64), dtype=np.float32)  # (c, k, l, c', s)
    Bm[0, :, :, 0, :] = np.transpose(w2r, (2, 0, 1))     # y_re rows, out re
    Bm[1, :, :, 0, :] = -np.transpose(w2i, (2, 0, 1))    # y_im rows, out re
    Bm[0, :, :, 1, :] = np.transpose(w2i, (2, 0, 1))     # y_re rows, out im
    Bm[1, :, :, 1, :] = np.transpose(w2r, (2, 0, 1))     # y_im rows, out im
    W2T = Bm.reshape(128, 64 * 128).astype(F16)
    return np.ascontiguousarray(W1T), np.ascontiguousarray(W2T)


def build_bass(repeat=1):
    import concourse.bacc as bacc
    import concourse.mybir as mybir
    import concourse.tile as tile

    f16 = mybir.dt.float16
    f32 = mybir.dt.float32

    nc = bacc.Bacc("TRN2", target_bir_lowering=False)
    x = nc.dram_tensor("x", [B_CORE, N], f32, kind="ExternalInput")
    w1 = nc.dram_tensor("w1", [128, 64 * 128], f16, kind="ExternalInput")
    w2 = nc.dram_tensor("w2", [128, 64 * 128], f16, kind="ExternalInput")
    iddram = nc.dram_tensor("ident", [128, 128], f16, kind="ExternalInput")
    out = nc.dram_tensor("out", [64, 128, B_CORE], f16, kind="ExternalOutput")

    x_v = x[:, :].rearrange("(t h b) n -> t h b n", h=2, b=128)
    # dst AP iterates (cs, L, hb) to match the OUTS source tile layout
    out_v = out[:, :, :].rearrange("L cs (t hb) -> t cs L hb", hb=BT)

    with tile.TileContext(nc) as tc:
        with (
            tc.tile_pool(name="const", bufs=1) as constp,
            tc.tile_pool(name="xs", bufs=4) as xsp,
            tc.tile_pool(name="t1", bufs=2) as t1p,
            tc.tile_pool(name="g", bufs=3) as gp,
            tc.tile_pool(name="t2", bufs=5) as t2p,
            tc.tile_pool(name="outs", bufs=5) as outp,
            tc.tile_pool(name="pt", bufs=3, space="PSUM") as ptp,
            tc.tile_pool(name="po", bufs=5, space="PSUM") as pop,
        ):
            # supertile-0 input first (PE's first dependency), then the weight
            # tables in interleaved chunks so stage 1 / stage 2 of supertile 0
            # don't wait on one monolithic load
            ident = constp.tile([128, 128], f16)
            nc.sync.dma_start(ident[:], iddram[:, :])
            XS0 = [xsp.tile([128, N], f16, tag="xs", name=f"xs0_{h}")
                   for h in range(2)]
            nc.gpsimd.dma_start(XS0[0][:], x_v[0][0])
            nc.gpsimd.dma_start(XS0[1][:], x_v[0][1])
            W1t = constp.tile([128, 64 * 128], f16)
            W2t = constp.tile([128, 64 * 128], f16)
            for ch in range(4):
                nc.sync.dma_start(W1t[:, ch * 2048:(ch + 1) * 2048],
                                  w1[:, ch * 2048:(ch + 1) * 2048])
            for ch in range(4):
                nc.sync.dma_start(W2t[:, ch * 2048:(ch + 1) * 2048],
                                  w2[:, ch * 2048:(ch + 1) * 2048])

            # greedy-balanced evac engine choice with anti-clumping so
            # consecutive copies land on different engines
            eng_load = {"v": 0.0, "s": 0.0, "p": 0.0}
            eng_fn = {"v": nc.vector.tensor_copy, "s": nc.scalar.copy,
                      "p": nc.gpsimd.tensor_copy}
            last_e = [None]

            def evac(dst, src, cols, kind):
                # kind: "sbuf16" = SBUF->SBUF fp16 (DVE 4x; Pool-legal),
                # "psum16" = PSUM fp16 src (DVE 2x), "psum32" = PSUM fp32 src.
                # GPSIMD must not touch PSUM (BIR rule), so Pool only gets
                # sbuf16 copies.
                vrate = {"sbuf16": 0.26, "psum16": 0.52, "psum32": 1.04}[kind]
                cost = {
                    "v": cols * vrate + 250,
                    "s": cols * 0.833 + 290,
                    "p": cols * 1.39 + 495,
                }
                cands = ["v", "s", "p"] if kind == "sbuf16" else ["v", "s"]
                e = min(cands, key=lambda k: eng_load[k] + cost[k]
                        + (300 if k == last_e[0] else 0))
                eng_load[e] += cost[e]
                last_e[0] = e
                eng_fn[e](dst, src)

            def permute_x(XS_h, XSP_h, g8):
                # (b, (p k)) -> (b, (j p dk)) column permute, SBUF->SBUF
                src_ap = XS_h[:].rearrange(
                    "b (p j dk) -> b j p dk", p=64, dk=2)[:, g8 * 8:g8 * 8 + 8]
                dst_ap = XSP_h[:].rearrange(
                    "b (j pdk) -> b j pdk", pdk=128)[:, g8 * 8:g8 * 8 + 8]
                dst_ap = dst_ap.rearrange("b j (p dk) -> b j p dk", dk=2)
                evac(dst_ap, src_ap, 1024, "sbuf16")

            def in_transpose(XSP_h, T1_h, g8, ident):
                P1 = ptp.tile([128, 8, 128], f16, tag="pt", name="P1")
                for jj in range(8):
                    j = g8 * 8 + jj
                    nc.tensor.transpose(
                        P1[:, jj, :], XSP_h[:, j * 128:(j + 1) * 128],
                        ident[:])
                evac(T1_h[:, g8 * 8:(g8 + 1) * 8, :], P1[:], 1024, "psum16")

            def stage1(T1_h, G_h, W1t, g2):
                O1 = pop.tile([128, 2, 256], f32, tag="po", name="O1")
                for jj in range(2):
                    j = g2 * 2 + jj
                    nc.tensor.matmul(
                        O1[:, jj, :], T1_h[:, j, :],
                        W1t[:, j * 256:(j + 1) * 256],
                        start=True, stop=True,
                    )
                # (j2, dk, c, q) -> (q, c, k=2*j2+dk)
                src = O1[:].rearrange("b j2 (dk c q) -> b q c j2 dk",
                                      dk=2, c=2)
                dst = G_h[:, :, :, 4 * g2:4 * g2 + 4].rearrange(
                    "b q c (j2 dk) -> b q c j2 dk", dk=2)
                evac(dst, src, 512, "psum32")

            from contextlib import nullcontext
            rep_ctx = tc.For_i(0, repeat, 1) if repeat > 1 else nullcontext()
            with rep_ctx:
                xs_tiles = {0: XS0}
                for t in range(NT):
                    # ---- input: contiguous cast DMA (fp32 -> fp16) ----
                    if t == 0 and repeat > 1:
                        XS = [xsp.tile([128, N], f16, tag="xs",
                                       name=f"xs_{h}") for h in range(2)]
                        for h in range(2):
                            nc.gpsimd.dma_start(XS[h][:], x_v[t][h])
                    else:
                        XS = xs_tiles.pop(t)
                    # prefetch next supertile before Pool gets busy with evacs
                    if t + 1 < NT:
                        XSn = [xsp.tile([128, N], f16, tag="xs",
                                        name=f"xsn_{h}") for h in range(2)]
                        for h in range(2):
                            nc.gpsimd.dma_start(XSn[h][:], x_v[t + 1][h])
                        xs_tiles[t + 1] = XSn

                    T1 = [t1p.tile([128, 32, 128], f16, tag="t1",
                                   name=f"t1_{h}") for h in range(2)]
                    G = [gp.tile([128, 64, 2, 64], f16, tag="g",
                                 name=f"g_{h}") for h in range(2)]
                    XSP = [xsp.tile([128, N], f16, tag="xsp", bufs=2,
                                    name=f"xsp_{h}") for h in range(2)]

                    # ---- column permute + input transpose + stage 1,
                    # software-pipelined per h: permute group feeds transpose
                    # group feeds 4 matmul groups (2 j's each).
                    for h in range(2):
                        permute_x(XS[h], XSP[h], 0)
                        permute_x(XS[h], XSP[h], 1)
                        in_transpose(XSP[h], T1[h], 0, ident)
                        for g2 in range(16):
                            if g2 in (0, 4):
                                permute_x(XS[h], XSP[h], g2 // 4 + 2)
                            if g2 in (0, 4, 8):
                                in_transpose(XSP[h], T1[h], g2 // 4 + 1, ident)
                            stage1(T1[h], G[h], W1t, g2)

                    # ---- middle transpose + stage 2, one lg-group ahead ----
                    T2s = {}

                    def mid_t(lg):
                        T2 = t2p.tile([128, 8, 256], f16, tag="t2",
                                      name="T2")
                        for h in range(2):
                            Gv = G[h][:].rearrange("b q c k -> b q (c k)")
                            P2 = ptp.tile([128, 8, 128], f16, tag="pt",
                                          name="P2")
                            for ll in range(8):
                                nc.tensor.transpose(
                                    P2[:, ll, :], Gv[:, lg * 8 + ll], ident[:])
                            evac(T2[:, :, h * 128:(h + 1) * 128], P2[:],
                                 1024, "psum16")
                        T2s[lg] = T2

                    mid_t(0)
                    for lg in range(8):
                        OUTS = outp.tile([128, 8, 256], f16, tag="outs",
                                         name="OUTS")
                        if lg + 1 < 8:
                            mid_t(lg + 1)
                        T2 = T2s.pop(lg)

                        for lq in range(4):
                            O2 = pop.tile([128, 2, 256], f32, tag="po",
                                          name="O2")
                            for ll in range(2):
                                l = lg * 8 + lq * 2 + ll
                                nc.tensor.matmul(
                                    O2[:, ll, :],
                                    W2t[:, l * 128:(l + 1) * 128],
                                    T2[:, lq * 2 + ll, :],
                                    start=True, stop=True,
                                )
                            evac(OUTS[:, lq * 2:lq * 2 + 2], O2[:],
                                 512, "psum32")

                        nc.sync.dma_start(
                            out_v[t][:, lg * 8:(lg + 1) * 8, :], OUTS[:])
    nc.compile()
    return nc


def _assemble_core(o: np.ndarray) -> np.ndarray:
    # o: (64 l, 128 (c' s), B_CORE) f16 -> (B_CORE, 4096) complex64
    a = o.reshape(64, 2, 64, B_CORE)                         # (l, c', s, b)
    a = np.ascontiguousarray(np.transpose(a, (3, 2, 0, 1)))  # (b, s, l, c')
    return a.astype(np.float32).view(np.complex64).reshape(B_CORE, N)


def kernel(x, w1_bfly, w2_bfly, perm, _trace=False):
    from concourse.bass_utils import run_bass_kernel_spmd

    x = np.asarray(x, dtype=np.float32)
    w1_bfly = np.asarray(w1_bfly, dtype=np.float32)
    w2_bfly = np.asarray(w2_bfly, dtype=np.float32)

    W1T, W2T = _build_host_weights(w1_bfly, w2_bfly)
    ident = np.eye(128, dtype=F16)
    nc = build_bass()
    in_maps = [
        {
            "x": np.ascontiguousarray(x[i * B_CORE:(i + 1) * B_CORE]),
            "w1": W1T,
            "w2": W2T,
            "ident": ident,
        }
        for i in range(NCORES)
    ]
    res = run_bass_kernel_spmd(
        nc, in_maps, core_ids=list(range(NCORES)), trace=_trace
    )
    outs = [_assemble_core(r["out"]) for r in res.results]
    full = np.concatenate(outs, axis=0)
    if _trace:
        return full, res
    return full


# revision 21
# speedup vs baseline: 1.0624x; 1.0095x over previous
"""Trainium2 Bass kernel for nn_ButterflyFFT (Monarch butterfly, N=4096, B=8192).

Math (per batch row b, with x[b] viewed as X[p,k] = x[b, p*64+k]):
  stage 1: out1[b,k,q] = sum_p w1c[k,q,p] X[p,k]          (complex, X real)
  stage 2: out2[b,l,s] = sum_k w2c[l,s,k] out1[b,k,l]     (complex)
  output:  out[b, s*64+l] = out2[b,l,s]

Device pipeline per core (B_core=1024, supertiles BT=256 = 2 halves h of 128):
  1. contiguous SWDGE cast-DMA x (fp32) -> XS[b0, (p k)] fp16
  2. input transpose on PE: XS[:, (p,dk)@j] -> T1[(p dk), b0] per k-pair j
     (partitions (p,dk)-interleaved; matches host weight row order)
  3. stage 1, k-pair block-diagonal matmuls: out O1[b0, (dk c q)] per (h,j)
  4. middle transpose on PE per (h,l): G[b0,(c k)] -> T2[(c k), b0]
  5. stage 2 weights-stationary: O2[(c' s), (h b0)] per l
  6. one output DMA per 8 l's; host reassembles complex64.
"""

import numpy as np

N = 4096
B = 8192
NCORES = 8
B_CORE = B // NCORES  # 1024
BT = 256              # supertile batch
NT = B_CORE // BT     # 4 supertiles
F16 = np.float16


def _build_host_weights(w1_bfly: np.ndarray, w2_bfly: np.ndarray):
    """W1T[p*2+dk, j*256 + dk'*128 + c*64 + q] = w1[2j+dk', q, p, c] * (dk==dk')
       W2T[c*64+k, l*128 + c'*64 + s] = stage-2 complex-matmul real form."""
    w1 = w1_bfly.astype(np.float32)                # (k, q, p, c)
    w1r = w1.reshape(32, 2, 64, 64, 2)             # (j, dk, q, p, c)
    A = np.zeros((64, 2, 32, 2, 2, 64), np.float32)  # (p, dk, j, dk', c, q)
    for dk in range(2):
        # (j, q, p, c) -> (p, j, c, q)
        A[:, dk, :, dk, :, :] = np.transpose(w1r[:, dk], (2, 0, 3, 1))
    W1T = A.reshape(128, 64 * 128).astype(F16)

    w2r = w2_bfly[..., 0].astype(np.float32)       # (l, s, k)
    w2i = w2_bfly[..., 1].astype(np.float32)
    Bm = np.empty((2, 64, 64, 2, 64), dtype=np.float32)  # (c, k, l, c', s)
    Bm[0, :, :, 0, :] = np.transpose(w2r, (2, 0, 1))     # y_re rows, out re
    Bm[1, :, :, 0, :] = -np.transpose(w2i, (2, 0, 1))    # y_im rows, out re
    Bm[0, :, :, 1, :] = np.transpose(w2i, (2, 0, 1))     # y_re rows, out im
    Bm[1, :, :, 1, :] = np.transpose(w2r, (2, 0, 1))     # y_im rows, out im
    W2T = Bm.reshape(128, 64 * 128).astype(F16)
    return np.ascontiguousarray(W1T), np.ascontiguousarray(W2T)


def build_bass(repeat=1):
    import concourse.bacc as bacc
    import concourse.mybir as mybir
    import concourse.tile as tile

    f16 = mybir.dt.float16
    f32 = mybir.dt.float32

    nc = bacc.Bacc("TRN2", target_bir_lowering=False)
    x = nc.dram_tensor("x", [B_CORE, N], f32, kind="ExternalInput")
    w1 = nc.dram_tensor("w1", [128, 64 * 128], f16, kind="ExternalInput")
    w2 = nc.dram_tensor("w2", [128, 64 * 128], f16, kind="ExternalInput")
    iddram = nc.dram_tensor("ident", [128, 128], f16, kind="ExternalInput")
    out = nc.dram_tensor("out", [64, 128, B_CORE], f16, kind="ExternalOutput")

    x_v = x[:, :].rearrange("(t h b) n -> t h b n", h=2, b=128)
    # dst AP iterates (cs, L, hb) to match the OUTS source tile layout
    out_v = out[:, :, :].rearrange("L cs (t hb) -> t cs L hb", hb=BT)

    with tile.TileContext(nc) as tc:
        with (
            tc.tile_pool(name="const", bufs=1) as constp,
            tc.tile_pool(name="xs", bufs=4) as xsp,
            tc.tile_pool(name="t1", bufs=2) as t1p,
            tc.tile_pool(name="g", bufs=3) as gp,
            tc.tile_pool(name="t2", bufs=5) as t2p,
            tc.tile_pool(name="outs", bufs=5) as outp,
            tc.tile_pool(name="pt", bufs=2, space="PSUM") as ptp,
            tc.tile_pool(name="po", bufs=3, space="PSUM") as pop,
        ):
            # supertile-0 input first (PE's first dependency), then the weight
            # tables in interleaved chunks so stage 1 / stage 2 of supertile 0
            # don't wait on one monolithic load
            ident = constp.tile([128, 128], f16)
            nc.sync.dma_start(ident[:], iddram[:, :])
            XS0 = [xsp.tile([128, N], f16, tag="xs", name=f"xs0_{h}")
                   for h in range(2)]
            nc.gpsimd.dma_start(XS0[0][:], x_v[0][0])
            nc.gpsimd.dma_start(XS0[1][:], x_v[0][1])
            W1t = constp.tile([128, 64 * 128], f16)
            W2t = constp.tile([128, 64 * 128], f16)
            for ch in range(4):
                nc.sync.dma_start(W1t[:, ch * 2048:(ch + 1) * 2048],
                                  w1[:, ch * 2048:(ch + 1) * 2048])
            for ch in range(4):
                nc.sync.dma_start(W2t[:, ch * 2048:(ch + 1) * 2048],
                                  w2[:, ch * 2048:(ch + 1) * 2048])

            # greedy-balanced evac engine choice with anti-clumping so
            # consecutive copies land on different engines
            eng_load = {"v": 0.0, "s": 0.0, "p": 0.0}
            eng_fn = {"v": nc.vector.tensor_copy, "s": nc.scalar.copy,
                      "p": nc.gpsimd.tensor_copy}
            last_e = [None]

            def evac(dst, src, cols, kind):
                # kind: "sbuf16" = SBUF->SBUF fp16 (DVE 4x; Pool-legal),
                # "psum16" = PSUM fp16 src (DVE 2x), "psum32" = PSUM fp32 src.
                # GPSIMD must not touch PSUM (BIR rule), so Pool only gets
                # sbuf16 copies.
                vrate = {"sbuf16": 0.26, "psum16": 0.52, "psum32": 1.04}[kind]
                cost = {
                    "v": cols * vrate + 250,
                    "s": cols * 0.833 + 290,
                    "p": cols * 1.39 + 495,
                }
                cands = ["v", "s", "p"] if kind == "sbuf16" else ["v", "s"]
                e = min(cands, key=lambda k: eng_load[k] + cost[k]
                        + (300 if k == last_e[0] else 0))
                eng_load[e] += cost[e]
                last_e[0] = e
                eng_fn[e](dst, src)

            def permute_x(XS_h, XSP_h, g8):
                # (b, (p k)) -> (b, (j p dk)) column permute, SBUF->SBUF
                src_ap = XS_h[:].rearrange(
                    "b (p j dk) -> b j p dk", p=64, dk=2)[:, g8 * 8:g8 * 8 + 8]
                dst_ap = XSP_h[:].rearrange(
                    "b (j pdk) -> b j pdk", pdk=128)[:, g8 * 8:g8 * 8 + 8]
                dst_ap = dst_ap.rearrange("b j (p dk) -> b j p dk", dk=2)
                evac(dst_ap, src_ap, 1024, "sbuf16")

            def in_transpose_xbar(XSP_h, T1_h):
                # XBAR: T1[(p dk), j, b0] = XSP[b0, j*128 + (p dk)]
                nc.sync.dma_start_transpose(T1_h[:], XSP_h[:])

            def in_transpose_pe(XSP_h, T1_h, g8):
                P1 = ptp.tile([128, 8, 128], f16, tag="pt", name="P1")
                for jj in range(8):
                    j = g8 * 8 + jj
                    nc.tensor.transpose(
                        P1[:, jj, :], XSP_h[:, j * 128:(j + 1) * 128],
                        ident[:])
                evac(T1_h[:, g8 * 8:(g8 + 1) * 8, :], P1[:], 1024, "psum16")

            def stage1(T1_h, G_h, W1t, g4):
                O1 = pop.tile([128, 4, 256], f32, tag="po", name="O1")
                for jj in range(4):
                    j = g4 * 4 + jj
                    nc.tensor.matmul(
                        O1[:, jj, :], T1_h[:, j, :],
                        W1t[:, j * 256:(j + 1) * 256],
                        start=True, stop=True,
                    )
                # (j4, dk, c, q) -> (q, c, k=2*j4+dk)
                src = O1[:].rearrange("b j4 (dk c q) -> b q c j4 dk",
                                      dk=2, c=2)
                dst = G_h[:, :, :, 8 * g4:8 * g4 + 8].rearrange(
                    "b q c (j4 dk) -> b q c j4 dk", dk=2)
                evac(dst, src, 1024, "psum32")

            def make_t1(t, XS_t, use_pe=False):
                # permutes (engines) + transpose (XBAR DMA, or PE for the
                # startup supertile) producing T1(t)
                T1 = [t1p.tile([128, 32, 128], f16, tag="t1",
                               name=f"t1_{h}") for h in range(2)]
                XSP = [xsp.tile([128, N], f16, tag="xsp", bufs=2,
                                name=f"xsp_{h}") for h in range(2)]
                for h in range(2):
                    for g8 in range(4):
                        permute_x(XS_t[h], XSP[h], g8)
                        if use_pe:
                            in_transpose_pe(XSP[h], T1[h], g8)
                    if not use_pe:
                        in_transpose_xbar(XSP[h], T1[h])
                return T1

            from contextlib import nullcontext
            rep_ctx = tc.For_i(0, repeat, 1) if repeat > 1 else nullcontext()
            with rep_ctx:
                xs_tiles = {0: XS0}
                t1_tiles = {}
                for t in range(NT):
                    if t == 0:
                        if repeat > 1:
                            XS = [xsp.tile([128, N], f16, tag="xs",
                                           name=f"xs_{h}") for h in range(2)]
                            for h in range(2):
                                nc.gpsimd.dma_start(XS[h][:], x_v[t][h])
                        else:
                            XS = xs_tiles.pop(0)
                        t1_tiles[0] = make_t1(0, XS, use_pe=True)
                        # first prefetch immediately (DMA is idle early)
                        XSn = [xsp.tile([128, N], f16, tag="xs",
                                        name=f"xsn_{h}") for h in range(2)]
                        for h in range(2):
                            nc.gpsimd.dma_start(XSn[h][:], x_v[1][h])
                        xs_tiles[1] = XSn

                    T1 = t1_tiles.pop(t)
                    G = [gp.tile([128, 64, 2, 64], f16, tag="g",
                                 name=f"g_{h}") for h in range(2)]

                    for h in range(2):
                        for g4 in range(8):
                            stage1(T1[h], G[h], W1t, g4)

                    # ---- middle transpose + stage 2, one lg-group ahead;
                    # next supertile's permutes+XBAR slot in mid-phase, after
                    # its input DMA has landed ----
                    T2s = {}

                    def mid_t(lg):
                        T2 = t2p.tile([128, 8, 256], f16, tag="t2",
                                      name="T2")
                        for h in range(2):
                            Gv = G[h][:].rearrange("b q c k -> b q (c k)")
                            P2 = ptp.tile([128, 8, 128], f16, tag="pt",
                                          name="P2")
                            for ll in range(8):
                                nc.tensor.transpose(
                                    P2[:, ll, :], Gv[:, lg * 8 + ll], ident[:])
                            evac(T2[:, :, h * 128:(h + 1) * 128], P2[:],
                                 1024, "psum16")
                        T2s[lg] = T2

                    mid_t(0)
                    for lg in range(8):
                        OUTS = outp.tile([128, 8, 256], f16, tag="outs",
                                         name="OUTS")
                        if lg + 1 < 8:
                            mid_t(lg + 1)
                        T2 = T2s.pop(lg)

                        for lq in range(2):
                            O2 = pop.tile([128, 4, 256], f32, tag="po",
                                          name="O2")
                            for ll in range(4):
                                l = lg * 8 + lq * 4 + ll
                                nc.tensor.matmul(
                                    O2[:, ll, :],
                                    W2t[:, l * 128:(l + 1) * 128],
                                    T2[:, lq * 4 + ll, :],
                                    start=True, stop=True,
                                )
                            evac(OUTS[:, lq * 4:lq * 4 + 4], O2[:],
                                 1024, "psum32")

                        nc.sync.dma_start(
                            out_v[t][:, lg * 8:(lg + 1) * 8, :], OUTS[:])

                        if lg == 2 and t + 1 < NT:
                            t1_tiles[t + 1] = make_t1(t + 1, xs_tiles[t + 1])
                        if lg == 5 and t + 2 < NT:
                            XSn = [xsp.tile([128, N], f16, tag="xs",
                                            name=f"xsn_{h}") for h in range(2)]
                            for h in range(2):
                                nc.gpsimd.dma_start(XSn[h][:], x_v[t + 2][h])
                            xs_tiles[t + 2] = XSn
    nc.compile()
    return nc


def _assemble_core(o: np.ndarray) -> np.ndarray:
    # o: (64 l, 128 (c' s), B_CORE) f16 -> (B_CORE, 4096) complex64
    a = o.reshape(64, 2, 64, B_CORE)                         # (l, c', s, b)
    a = np.ascontiguousarray(np.transpose(a, (3, 2, 0, 1)))  # (b, s, l, c')
    return a.astype(np.float32).view(np.complex64).reshape(B_CORE, N)


def kernel(x, w1_bfly, w2_bfly, perm, _trace=False):
    from concourse.bass_utils import run_bass_kernel_spmd

    x = np.asarray(x, dtype=np.float32)
    w1_bfly = np.asarray(w1_bfly, dtype=np.float32)
    w2_bfly = np.asarray(w2_bfly, dtype=np.float32)

    W1T, W2T = _build_host_weights(w1_bfly, w2_bfly)
    ident = np.eye(128, dtype=F16)
    nc = build_bass()
    in_maps = [
        {
            "x": np.ascontiguousarray(x[i * B_CORE:(i + 1) * B_CORE]),
            "w1": W1T,
            "w2": W2T,
            "ident": ident,
        }
        for i in range(NCORES)
    ]
    res = run_bass_kernel_spmd(
        nc, in_maps, core_ids=list(range(NCORES)), trace=_trace
    )
    outs = [_assemble_core(r["out"]) for r in res.results]
    full = np.concatenate(outs, axis=0)
    if _trace:
        return full, res
    return full


# revision 23
# speedup vs baseline: 1.0625x; 1.0002x over previous
"""Trainium2 Bass kernel for nn_ButterflyFFT (Monarch butterfly, N=4096, B=8192).

Math (per batch row b, with x[b] viewed as X[p,k] = x[b, p*64+k]):
  stage 1: out1[b,k,q] = sum_p w1c[k,q,p] X[p,k]          (complex, X real)
  stage 2: out2[b,l,s] = sum_k w2c[l,s,k] out1[b,k,l]     (complex)
  output:  out[b, s*64+l] = out2[b,l,s]

Device pipeline per core (B_core=1024, supertiles BT=256 = 2 halves h of 128):
  1. contiguous SWDGE cast-DMA x (fp32) -> XS[b0, (p k)] fp16
  2. input transpose on PE: XS[:, (p,dk)@j] -> T1[(p dk), b0] per k-pair j
     (partitions (p,dk)-interleaved; matches host weight row order)
  3. stage 1, k-pair block-diagonal matmuls: out O1[b0, (dk c q)] per (h,j)
  4. middle transpose on PE per (h,l): G[b0,(c k)] -> T2[(c k), b0]
  5. stage 2 weights-stationary: O2[(c' s), (h b0)] per l
  6. one output DMA per 8 l's; host reassembles complex64.
"""

import numpy as np

N = 4096
B = 8192
NCORES = 8
B_CORE = B // NCORES  # 1024
BT = 256              # supertile batch
NT = B_CORE // BT     # 4 supertiles
F16 = np.float16


def _build_host_weights(w1_bfly: np.ndarray, w2_bfly: np.ndarray):
    """W1T[p*2+dk, j*256 + dk'*128 + c*64 + q] = w1[2j+dk', q, p, c] * (dk==dk')
       W2T[c*64+k, l*128 + c'*64 + s] = stage-2 complex-matmul real form."""
    w1 = w1_bfly.astype(np.float32)                # (k, q, p, c)
    w1r = w1.reshape(32, 2, 64, 64, 2)             # (j, dk, q, p, c)
    A = np.zeros((64, 2, 32, 2, 2, 64), np.float32)  # (p, dk, j, dk', c, q)
    for dk in range(2):
        # (j, q, p, c) -> (p, j, c, q)
        A[:, dk, :, dk, :, :] = np.transpose(w1r[:, dk], (2, 0, 3, 1))
    W1T = A.reshape(128, 64 * 128).astype(F16)

    w2r = w2_bfly[..., 0].astype(np.float32)       # (l, s, k)
    w2i = w2_bfly[..., 1].astype(np.float32)
    Bm = np.empty((2, 64, 64, 2, 64), dtype=np.float32)  # (c, k, l, c', s)
    Bm[0, :, :, 0, :] = np.transpose(w2r, (2, 0, 1))     # y_re rows, out re
    Bm[1, :, :, 0, :] = -np.transpose(w2i, (2, 0, 1))    # y_im rows, out re
    Bm[0, :, :, 1, :] = np.transpose(w2i, (2, 0, 1))     # y_re rows, out im
    Bm[1, :, :, 1, :] = np.transpose(w2r, (2, 0, 1))     # y_im rows, out im
    W2T = Bm.reshape(128, 64 * 128).astype(F16)
    return np.ascontiguousarray(W1T), np.ascontiguousarray(W2T)


def build_bass(repeat=1):
    import concourse.bacc as bacc
    import concourse.mybir as mybir
    import concourse.tile as tile

    f16 = mybir.dt.float16
    f32 = mybir.dt.float32

    nc = bacc.Bacc("TRN2", target_bir_lowering=False)
    x = nc.dram_tensor("x", [B_CORE, N], f32, kind="ExternalInput")
    w1 = nc.dram_tensor("w1", [128, 64 * 128], f16, kind="ExternalInput")
    w2 = nc.dram_tensor("w2", [128, 64 * 128], f16, kind="ExternalInput")
    iddram = nc.dram_tensor("ident", [128, 128], f16, kind="ExternalInput")
    out = nc.dram_tensor("out", [64, 128, B_CORE], f16, kind="ExternalOutput")

    x_v = x[:, :].rearrange("(t h b) n -> t h b n", h=2, b=128)
    # dst AP iterates (cs, L, hb) to match the OUTS source tile layout
    out_v = out[:, :, :].rearrange("L cs (t hb) -> t cs L hb", hb=BT)

    with tile.TileContext(nc) as tc:
        with (
            tc.tile_pool(name="const", bufs=1) as constp,
            tc.tile_pool(name="xs", bufs=4) as xsp,
            tc.tile_pool(name="t1", bufs=2) as t1p,
            tc.tile_pool(name="g", bufs=3) as gp,
            tc.tile_pool(name="t2", bufs=5) as t2p,
            tc.tile_pool(name="outs", bufs=5) as outp,
            tc.tile_pool(name="pt", bufs=2, space="PSUM") as ptp,
            tc.tile_pool(name="po", bufs=3, space="PSUM") as pop,
        ):
            # supertile-0 input first (PE's first dependency), then the weight
            # tables in interleaved chunks so stage 1 / stage 2 of supertile 0
            # don't wait on one monolithic load
            ident = constp.tile([128, 128], f16)
            nc.sync.dma_start(ident[:], iddram[:, :])
            XS0 = [xsp.tile([128, N], f16, tag="xs", name=f"xs0_{h}")
                   for h in range(2)]
            nc.gpsimd.dma_start(XS0[0][:], x_v[0][0])
            nc.gpsimd.dma_start(XS0[1][:], x_v[0][1])
            W1t = constp.tile([128, 64 * 128], f16)
            W2t = constp.tile([128, 64 * 128], f16)
            for ch in range(4):
                nc.sync.dma_start(W1t[:, ch * 2048:(ch + 1) * 2048],
                                  w1[:, ch * 2048:(ch + 1) * 2048])
            for ch in range(4):
                nc.sync.dma_start(W2t[:, ch * 2048:(ch + 1) * 2048],
                                  w2[:, ch * 2048:(ch + 1) * 2048])

            # greedy-balanced evac engine choice with anti-clumping so
            # consecutive copies land on different engines
            eng_load = {"v": 0.0, "s": 0.0, "p": 0.0}
            eng_fn = {"v": nc.vector.tensor_copy, "s": nc.scalar.copy,
                      "p": nc.gpsimd.tensor_copy}
            last_e = [None]

            def evac(dst, src, cols, kind):
                # kind: "sbuf16" = SBUF->SBUF fp16 (DVE 4x; Pool-legal),
                # "psum16" = PSUM fp16 src (DVE 2x), "psum32" = PSUM fp32 src.
                # GPSIMD must not touch PSUM (BIR rule), so Pool only gets
                # sbuf16 copies.
                vrate = {"sbuf16": 0.26, "psum16": 0.52, "psum32": 1.04}[kind]
                cost = {
                    "v": cols * vrate + 250,
                    "s": cols * 0.833 + 290,
                    "p": cols * 1.39 + 495,
                }
                cands = ["v", "s", "p"] if kind == "sbuf16" else ["v", "s"]
                e = min(cands, key=lambda k: eng_load[k] + cost[k]
                        + (300 if k == last_e[0] else 0))
                eng_load[e] += cost[e]
                last_e[0] = e
                eng_fn[e](dst, src)

            def permute_x(XS_h, XSP_h, g8):
                # (b, (p k)) -> (b, (j p dk)) column permute, SBUF->SBUF
                src_ap = XS_h[:].rearrange(
                    "b (p j dk) -> b j p dk", p=64, dk=2)[:, g8 * 8:g8 * 8 + 8]
                dst_ap = XSP_h[:].rearrange(
                    "b (j pdk) -> b j pdk", pdk=128)[:, g8 * 8:g8 * 8 + 8]
                dst_ap = dst_ap.rearrange("b j (p dk) -> b j p dk", dk=2)
                evac(dst_ap, src_ap, 1024, "sbuf16")

            def in_transpose_xbar(XSP_h, T1_h):
                # XBAR: T1[(p dk), j, b0] = XSP[b0, j*128 + (p dk)]
                nc.sync.dma_start_transpose(T1_h[:], XSP_h[:])

            def in_transpose_pe(XSP_h, T1_h, g8):
                P1 = ptp.tile([128, 8, 128], f16, tag="pt", name="P1")
                for jj in range(8):
                    j = g8 * 8 + jj
                    nc.tensor.transpose(
                        P1[:, jj, :], XSP_h[:, j * 128:(j + 1) * 128],
                        ident[:])
                evac(T1_h[:, g8 * 8:(g8 + 1) * 8, :], P1[:], 1024, "psum16")

            def stage1(T1_h, G_h, W1t, g4):
                O1 = pop.tile([128, 4, 256], f32, tag="po", name="O1")
                for jj in range(4):
                    j = g4 * 4 + jj
                    nc.tensor.matmul(
                        O1[:, jj, :], T1_h[:, j, :],
                        W1t[:, j * 256:(j + 1) * 256],
                        start=True, stop=True,
                    )
                # (j4, dk, c, q) -> (q, c, k=2*j4+dk)
                src = O1[:].rearrange("b j4 (dk c q) -> b q c j4 dk",
                                      dk=2, c=2)
                dst = G_h[:, :, :, 8 * g4:8 * g4 + 8].rearrange(
                    "b q c (j4 dk) -> b q c j4 dk", dk=2)
                evac(dst, src, 1024, "psum32")

            def make_t1(t, XS_t, use_pe=False):
                # permutes (engines) + transpose (XBAR DMA, or PE for the
                # startup supertile) producing T1(t)
                T1 = [t1p.tile([128, 32, 128], f16, tag="t1",
                               name=f"t1_{h}") for h in range(2)]
                XSP = [xsp.tile([128, N], f16, tag="xsp", bufs=2,
                                name=f"xsp_{h}") for h in range(2)]
                for h in range(2):
                    for g8 in range(4):
                        permute_x(XS_t[h], XSP[h], g8)
                        if use_pe:
                            in_transpose_pe(XSP[h], T1[h], g8)
                    if not use_pe:
                        in_transpose_xbar(XSP[h], T1[h])
                return T1

            from contextlib import nullcontext
            rep_ctx = tc.For_i(0, repeat, 1) if repeat > 1 else nullcontext()
            with rep_ctx:
                xs_tiles = {0: XS0}
                t1_tiles = {}
                for t in range(NT):
                    if t == 0:
                        if repeat > 1:
                            XS = [xsp.tile([128, N], f16, tag="xs",
                                           name=f"xs_{h}") for h in range(2)]
                            for h in range(2):
                                nc.gpsimd.dma_start(XS[h][:], x_v[t][h])
                        else:
                            XS = xs_tiles.pop(0)
                        t1_tiles[0] = make_t1(0, XS, use_pe=True)
                        # first prefetch immediately (DMA is idle early)
                        XSn = [xsp.tile([128, N], f16, tag="xs",
                                        name=f"xsn_{h}") for h in range(2)]
                        for h in range(2):
                            nc.gpsimd.dma_start(XSn[h][:], x_v[1][h])
                        xs_tiles[1] = XSn

                    T1 = t1_tiles.pop(t)
                    G = [gp.tile([128, 64, 2, 64], f16, tag="g",
                                 name=f"g_{h}") for h in range(2)]

                    for h in range(2):
                        for g4 in range(8):
                            stage1(T1[h], G[h], W1t, g4)

                    # ---- middle transpose + stage 2, one lg-group ahead;
                    # next supertile's permutes+XBAR slot in mid-phase, after
                    # its input DMA has landed ----
                    T2s = {}

                    def mid_t(lg):
                        T2 = t2p.tile([128, 8, 256], f16, tag="t2",
                                      name="T2")
                        for h in range(2):
                            Gv = G[h][:].rearrange("b q c k -> b q (c k)")
                            P2 = ptp.tile([128, 8, 128], f16, tag="pt",
                                          name="P2")
                            for ll in range(8):
                                nc.tensor.transpose(
                                    P2[:, ll, :], Gv[:, lg * 8 + ll], ident[:])
                            evac(T2[:, :, h * 128:(h + 1) * 128], P2[:],
                                 1024, "psum16")
                        T2s[lg] = T2

                    mid_t(0)
                    for lg in range(8):
                        OUTS = outp.tile([128, 8, 256], f16, tag="outs",
                                         name="OUTS")
                        if lg + 1 < 8:
                            mid_t(lg + 1)
                        T2 = T2s.pop(lg)

                        for lq in range(2):
                            O2 = pop.tile([128, 4, 256], f32, tag="po",
                                          name="O2")
                            for ll in range(4):
                                l = lg * 8 + lq * 4 + ll
                                nc.tensor.matmul(
                                    O2[:, ll, :],
                                    W2t[:, l * 128:(l + 1) * 128],
                                    T2[:, lq * 4 + ll, :],
                                    start=True, stop=True,
                                )
                            evac(OUTS[:, lq * 4:lq * 4 + 4], O2[:],
                                 1024, "psum32")

                        nc.sync.dma_start(
                            out_v[t][:, lg * 8:(lg + 1) * 8, :], OUTS[:])

                        if lg == 0 and t + 1 < NT:
                            t1_tiles[t + 1] = make_t1(t + 1, xs_tiles[t + 1])
                        if lg == 5 and t + 2 < NT:
                            XSn = [xsp.tile([128, N], f16, tag="xs",
                                            name=f"xsn_{h}") for h in range(2)]
                            for h in range(2):
                                nc.gpsimd.dma_start(XSn[h][:], x_v[t + 2][h])
                            xs_tiles[t + 2] = XSn
    nc.compile()
    return nc


def _assemble_core(o: np.ndarray) -> np.ndarray:
    # o: (64 l, 128 (c' s), B_CORE) f16 -> (B_CORE, 4096) complex64
    a = o.reshape(64, 2, 64, B_CORE)                         # (l, c', s, b)
    a = np.ascontiguousarray(np.transpose(a, (3, 2, 0, 1)))  # (b, s, l, c')
    return a.astype(np.float32).view(np.complex64).reshape(B_CORE, N)


def kernel(x, w1_bfly, w2_bfly, perm, _trace=False):
    from concourse.bass_utils import run_bass_kernel_spmd

    x = np.asarray(x, dtype=np.float32)
    w1_bfly = np.asarray(w1_bfly, dtype=np.float32)
    w2_bfly = np.asarray(w2_bfly, dtype=np.float32)

    W1T, W2T = _build_host_weights(w1_bfly, w2_bfly)
    ident = np.eye(128, dtype=F16)
    nc = build_bass()
    in_maps = [
        {
            "x": np.ascontiguousarray(x[i * B_CORE:(i + 1) * B_CORE]),
            "w1": W1T,
            "w2": W2T,
            "ident": ident,
        }
        for i in range(NCORES)
    ]
    res = run_bass_kernel_spmd(
        nc, in_maps, core_ids=list(range(NCORES)), trace=_trace
    )
    outs = [_assemble_core(r["out"]) for r in res.results]
    full = np.concatenate(outs, axis=0)
    if _trace:
        return full, res
    return full
